# revision 22
# baseline (speedup 1.0000x reference)
"""LogitLinear Trainium2 kernel: softmax-moment weights + dual GEMM, fp8.

out[n, 0, o] = sum_i mean(W_logits[:, o, i]) * x[n, i]   + mean(b_logits[:, o])
out[n, 1, o] = sum_i var(W_logits[:, o, i])  * x[n, i]^2 + var(b_logits[:, o])

Softmax over D=3 values [-1, 0, 1]. With a = e^{l2-l1}, b = e^{l0-l1}:
  E[w]   = (a-b)/(1+a+b) = sigmoid(z) * tanh(h/2)
  E[w^2] = (a+b)/(1+a+b) = sigmoid(z)
  Var[w] = E[w^2] - E[w]^2
where z = logaddexp(l2, l0) - l1 and h = l2 - l0. The host ships the
(z, h) reparametrization of the logits in fp8; the device evaluates the
softmax moments via ACT sigmoid/tanh tables (the softmax division lives
inside sigmoid), squares, subtracts, and runs both GEMMs as fp8e4
DoubleRow matmuls (157 TF/s). Bias moments are computed on-device the
same way and folded into the PSUM accumulation as a K=1 matmul.

Sharding: out_feat split across 8 cores (512 each); x replicated.
Host pre-transposes/pre-casts (x, x^2 in fp8; output returned bf16 and
upcast on host). The var channel dominates the output norm ~75:1, and
its GEMM sums positive terms, so fp8 quantization noise averages out
(measured ~3e-3 combined rel err vs the 2e-2 gate).

Schedule: 4 PSUM passes over n (4 n-tiles x {mean,var} = 8 banks).
Pass 0 computes the weight moments just-in-time per kq-pair while its
GEMMs consume them; x/x^2 arrive as per-pass n-quadrant slabs (JIT for
quadrant 0, three big background DMAs for quadrants 1-3 issued at
points chosen to keep pass-0's JIT traffic inside the HBM budget).
Moments are processed two kq at a time (FD=2048) to amortize per-op
overhead; all DVE inputs stay bf16 (fp8 reads on DVE cost ~2.7x).
"""

import numpy as np
import ml_dtypes

N, IN, OUT, D = 2048, 4096, 4096, 3
NCORES = 8
OS = OUT // NCORES  # 512 out-features per core
PAIR = 2            # k-tiles per DoubleRow matmul
KQ = IN // (128 * PAIR)  # 16 contraction pair-blocks
KQ2 = KQ // 2       # moment-pipeline granularity: 2 kq (512 i-rows) per step
NT = N // 128       # 16 n-tiles
NTQ = 4             # n-tiles per PSUM pass (8 banks = 4 mean + 4 var)
NPASS = NT // NTQ
NQ = N // NPASS     # 512 n-columns per pass quadrant
SKEW = 2            # kq-skew of var matmuls behind mean (wv8 is late in chain)

F8 = ml_dtypes.float8_e4m3
BF16 = ml_dtypes.bfloat16

_CACHED_NC = None


def _build():
    global _CACHED_NC
    if _CACHED_NC is not None:
        return _CACHED_NC
    import concourse.bass as bass
    import concourse.bacc as bacc
    import concourse.mybir as mybir
    import concourse.tile as tile

    dt = mybir.dt
    f32, bf16, fp8 = dt.float32, dt.bfloat16, dt.float8e4
    Sigmoid = mybir.ActivationFunctionType.Sigmoid
    Tanh = mybir.ActivationFunctionType.Tanh
    DR = mybir.MatmulPerfMode.DoubleRow

    nc = bacc.Bacc("TRN2", debug=False, num_devices=NCORES)
    wzh = nc.dram_tensor("wzh", [KQ2, 128, 2, 4, OS], fp8, kind="ExternalInput")
    xt8 = nc.dram_tensor("xt8", [KQ, NPASS, 128, PAIR, NQ], fp8, kind="ExternalInput")
    xq8 = nc.dram_tensor("xq8", [KQ, NPASS, 128, PAIR, NQ], fp8, kind="ExternalInput")
    bzh = nc.dram_tensor("bzh", [1, 2, OS], bf16, kind="ExternalInput")
    out = nc.dram_tensor("out", [NT, 128, 2, OS], bf16, kind="ExternalOutput")

    wzh_ap = wzh.ap()
    xt8_ap = xt8.ap()
    xq8_ap = xq8.ap()
    # quadrant-major views for the bulk background loads, dim-ordered to
    # match the SBUF tile layout [128, KQ, PAIR, NQ]
    xtq_ap = xt8.ap().rearrange("kq q p pair n -> q p kq pair n")
    xqq_ap = xq8.ap().rearrange("kq q p pair n -> q p kq pair n")
    out_ap = out.ap()

    with tile.TileContext(nc) as tc:
        with (
            tc.tile_pool(name="big", bufs=1) as big,
            tc.tile_pool(name="ld", bufs=2) as ld,
            tc.tile_pool(name="mt", bufs=2) as mt,
            tc.tile_pool(name="st", bufs=3) as st,
            tc.tile_pool(name="misc", bufs=1) as misc,
            tc.tile_pool(name="ps", bufs=8, space="PSUM") as ps,
        ):
            # double-buffered n-quadrant slots: pass p reads slot p%2 while the
            # next quadrant streams into slot (p+1)%2
            x8 = big.tile([128, KQ, 2, PAIR, NQ], fp8, tag="x8")
            xx8 = big.tile([128, KQ, 2, PAIR, NQ], fp8, tag="xx8")
            wm8 = big.tile([128, KQ2, 4, OS], fp8, tag="wm8")
            wv8 = big.tile([128, KQ2, 4, OS], fp8, tag="wv8")

            # warm the ACT sigmoid/tanh table set entirely on the scalar
            # queue (no cross-engine dep) so the table is loaded before the
            # first real sigmoid
            warm = misc.tile([1, 8], f32, tag="warm")
            nc.scalar.memzero(warm)
            nc.scalar.activation(out=warm, in_=warm, func=Sigmoid)

            # ---- bias moments (tiny, partition dim 1) + ones for K=1 mm ----
            # Declared here, emitted mid-pass-0 (off the moment critical path;
            # only needed by the first bias matmul at the end of pass 0).
            ones8 = misc.tile([1, 2, 128], fp8, tag="ones8")
            bias_m = misc.tile([1, 2, OS], fp8, tag="bias_m")
            bias_v = misc.tile([1, 2, OS], fp8, tag="bias_v")

            def emit_bias():
                ones_f = misc.tile([1, 2, 128], bf16, tag="ones_f")
                nc.vector.memset(ones_f, 1.0)
                nc.vector.tensor_copy(ones8, ones_f)
                bzh_t = misc.tile([1, 2, OS], bf16, tag="bzh_t")
                nc.sync.dma_start(out=bzh_t, in_=bzh.ap()[0])
                bE2 = misc.tile([1, OS], bf16, tag="bE2")
                nc.scalar.activation(out=bE2, in_=bzh_t[:, 0], func=Sigmoid)
                bt = misc.tile([1, OS], bf16, tag="bt")
                nc.scalar.activation(out=bt, in_=bzh_t[:, 1], func=Tanh, scale=0.5)
                nc.vector.memset(bias_m, 0.0)
                nc.vector.memset(bias_v, 0.0)
                bm_f = misc.tile([1, OS], bf16, tag="bm_f")
                nc.vector.tensor_mul(bm_f, bE2, bt)
                nc.vector.tensor_copy(bias_m[:, 0, :], bm_f)
                bm2 = misc.tile([1, OS], bf16, tag="bm2")
                nc.vector.tensor_mul(bm2, bm_f, bm_f)
                nc.vector.tensor_sub(bias_v[:, 0, :], bE2, bm2)

            # ---- per-kq-pair weight moments (FD = 2048) ----
            # All W tiles prefetched at t=0 on the gpsimd queue (idle until
            # the first wv8) so the moment pipeline is never DMA-issue-gated.
            wts = []
            for kq2 in range(KQ2):
                wt = ld.tile([128, 2, 4, OS], fp8, tag="wt", bufs=KQ2, name=f"wt{kq2}")
                nc.gpsimd.dma_start(out=wt, in_=wzh_ap[kq2])
                wts.append(wt)

            def emit_moments(kq2, halves=False):
                if halves:
                    # head of the pipeline: emit the two kq-halves of this
                    # pair separately so the first matmul's rhs is ready
                    # after an FD-1024 chain instead of FD-2048
                    wt = wts[kq2]
                    for hh in range(2):
                        s = slice(2 * hh, 2 * hh + 2)
                        E2h = mt.tile([128, 2, OS], bf16, tag="E2h", bufs=1)
                        nc.scalar.activation(out=E2h, in_=wt[:, 0, s], func=Sigmoid)
                        thh = mt.tile([128, 2, OS], bf16, tag="thh", bufs=1)
                        nc.scalar.activation(out=thh, in_=wt[:, 1, s], func=Tanh, scale=0.5)
                        wmh = mt.tile([128, 2, OS], bf16, tag="wmh", bufs=1)
                        nc.vector.tensor_mul(wmh, E2h, thh)
                        nc.vector.tensor_copy(wm8[:, kq2, s], wmh)
                        m2h = mt.tile([128, 2, OS], bf16, tag="m2h", bufs=1)
                        nc.vector.tensor_mul(m2h, wmh, wmh)
                        nc.gpsimd.tensor_sub(wv8[:, kq2, s], E2h, m2h)
                    return
                wt = wts[kq2]
                E2 = mt.tile([128, 4, OS], bf16, tag="E2", bufs=4)
                nc.scalar.activation(out=E2, in_=wt[:, 0], func=Sigmoid)
                th = mt.tile([128, 4, OS], bf16, tag="th", bufs=3)
                nc.scalar.activation(out=th, in_=wt[:, 1], func=Tanh, scale=0.5)
                wm_bf = mt.tile([128, 4, OS], bf16, tag="wm_bf", bufs=3)
                nc.vector.tensor_mul(wm_bf, E2, th)
                nc.vector.tensor_copy(wm8[:, kq2, 0:2], wm_bf[:, 0:2])
                nc.scalar.copy(wm8[:, kq2, 2:4], wm_bf[:, 2:4])
                m2 = mt.tile([128, 4, OS], bf16, tag="m2", bufs=3)
                nc.vector.tensor_mul(m2, wm_bf, wm_bf)
                nc.gpsimd.tensor_sub(wv8[:, kq2], E2, m2)

            for p in range(NPASS):
                first = p == 0
                nts = range(p * NTQ, (p + 1) * NTQ)
                psm = [
                    ps.tile([128, OS], f32, tag="ps", name=f"psm{p}_{j}")
                    for j in range(NTQ)
                ]
                psv = [
                    ps.tile([128, OS], f32, tag="ps", name=f"psv{p}_{j}")
                    for j in range(NTQ)
                ]

                def w_slice(w, kq):
                    h = kq % 2
                    return w[:, kq // 2, 2 * h : 2 * h + 2, :]

                def mean_mms(kq):
                    for j in range(NTQ):
                        nc.tensor.matmul(
                            psm[j],
                            lhsT=x8[:, kq, p % 2, :, j * 128 : (j + 1) * 128],
                            rhs=w_slice(wm8, kq),
                            start=(kq == 0),
                            stop=False,
                            perf_mode=DR,
                        )

                def var_mms(kq):
                    for j in range(NTQ):
                        nc.tensor.matmul(
                            psv[j],
                            lhsT=xx8[:, kq, p % 2, :, j * 128 : (j + 1) * 128],
                            rhs=w_slice(wv8, kq),
                            start=(kq == 0),
                            stop=False,
                            perf_mode=DR,
                        )

                if p < NPASS - 1:
                    # background bulk load of the NEXT pass's n-quadrant
                    # (issued at the start of this pass; for pass 0 the
                    # issue point is deferred into the kq loop below so the
                    # JIT quadrant-0 traffic keeps HBM priority early on)
                    def load_next_quadrant():
                        nc.sync.dma_start(
                            out=x8[:, :, (p + 1) % 2], in_=xtq_ap[p + 1]
                        )
                        nc.sync.dma_start(
                            out=xx8[:, :, (p + 1) % 2], in_=xqq_ap[p + 1]
                        )

                    if not first:
                        load_next_quadrant()

                for kq in range(KQ):
                    if first:
                        if kq % 2 == 0:
                            emit_moments(kq // 2)
                            nc.sync.dma_start(
                                out=x8[:, kq : kq + 2, 0],
                                in_=xtq_ap[0][:, kq : kq + 2],
                            )
                            nc.sync.dma_start(
                                out=xx8[:, kq : kq + 2, 0],
                                in_=xqq_ap[0][:, kq : kq + 2],
                            )
                        if kq == 5:
                            emit_bias()
                        if kq == 9:
                            load_next_quadrant()
                    mean_mms(kq)
                    kqv = kq - SKEW if first else kq
                    if kqv >= 0:
                        var_mms(kqv)
                if first:
                    for kq in range(KQ - SKEW, KQ):
                        var_mms(kq)
                for j, nt in enumerate(nts):
                    nc.tensor.matmul(
                        psm[j], lhsT=ones8, rhs=bias_m,
                        start=False, stop=True, perf_mode=DR,
                    )
                    nc.tensor.matmul(
                        psv[j], lhsT=ones8, rhs=bias_v,
                        start=False, stop=True, perf_mode=DR,
                    )
                    stg = st.tile([128, 2, OS], bf16, tag="stg")
                    # alternate drain engines: ACT near PSUM, DVE for the rest
                    if j % 2 == 0:
                        nc.scalar.copy(stg[:, 0, :], psm[j])
                        nc.vector.tensor_copy(stg[:, 1, :], psv[j])
                    else:
                        nc.vector.tensor_copy(stg[:, 0, :], psm[j])
                        nc.scalar.copy(stg[:, 1, :], psv[j])
                    nc.sync.dma_start(out=out_ap[nt], in_=stg)

    nc.compile()
    _CACHED_NC = nc
    return nc


def _to8(v):
    return np.clip(v, -240.0, 240.0).astype(F8)


def _wshuf(a):
    """[IN, OS] -> [KQ2, 128, 4, OS] with i = kq2*512 + m*128 + p."""
    return np.ascontiguousarray(
        a.reshape(KQ2, 4, 128, a.shape[1]).transpose(0, 2, 1, 3)
    )


def _xshuf(a):
    """[IN, N] -> [KQ, NPASS, 128, PAIR, NQ] with i = kq*256 + pair*128 + p,
    n = q*NQ + nn."""
    return np.ascontiguousarray(
        a.reshape(KQ, PAIR, 128, NPASS, NQ).transpose(0, 3, 2, 1, 4)
    )


def prep_inputs(x, W_logits, b_logits):
    """Host-side layout/precision prep. Returns per-core input maps."""
    x = np.asarray(x, dtype=np.float32)
    W_logits = np.asarray(W_logits, dtype=np.float32)
    b_logits = np.asarray(b_logits, dtype=np.float32)

    l0, l1, l2 = W_logits[0], W_logits[1], W_logits[2]  # (OUT, IN)
    z = np.logaddexp(l2, l0) - l1
    h = l2 - l0
    zT8 = _to8(z.T)  # (IN, OUT)
    hT8 = _to8(h.T)

    xt8 = _xshuf(_to8(x.T))
    xq8 = _xshuf(_to8((x * x).T))

    b0, b1, b2 = b_logits[0, :, 0], b_logits[1, :, 0], b_logits[2, :, 0]
    zb = np.logaddexp(b2, b0) - b1
    hb = b2 - b0

    in_maps = []
    for c in range(NCORES):
        sl = slice(c * OS, (c + 1) * OS)
        wzh_c = np.ascontiguousarray(
            np.stack([_wshuf(zT8[:, sl]), _wshuf(hT8[:, sl])], axis=2)
        )  # (KQ2, 128, 2, 4, OS)
        bzh_c = np.ascontiguousarray(
            np.stack([zb[sl], hb[sl]])[None].astype(BF16)
        )  # (1, 2, OS)
        in_maps.append({"wzh": wzh_c, "xt8": xt8, "xq8": xq8, "bzh": bzh_c})
    return in_maps


def collect_output(results):
    """Per-core bf16 [NT, 128, 2, OS] tiles -> full f32 (N, 2, OUT)."""
    full = np.empty((N, 2, OUT), dtype=np.float32)
    for c in range(NCORES):
        full[:, :, c * OS : (c + 1) * OS] = (
            results[c]["out"].astype(np.float32).reshape(N, 2, OS)
        )
    return full


def kernel(x, W_logits, b_logits):
    from concourse import bass_utils

    nc = _build()
    in_maps = prep_inputs(x, W_logits, b_logits)
    res = bass_utils.run_bass_kernel_spmd(
        nc, in_maps, core_ids=list(range(NCORES))
    )
    return collect_output(res.results)
